# revision 42
# baseline (speedup 1.0000x reference)
"""BitwiseWavenet Trainium2 kernel: 8-core SPMD, sequence-parallel sharding.

Layout: 4 partition groups of 32 channels = the 4 batches; L split 8 ways
across cores, each core computing a halo-extended window of W=10238 samples.
All convs are PE matmuls with block-diagonal (per-group) weights.

v8: fp8e4m3 residual stream / filter/gate/res/skip weights / layer outputs
(lo). fp8 DoubleRow matmuls contract both dilation taps of f/g in one PE
pass: large-dilation layers (d>=8) read the two taps as a strided k-tile
pair directly from the residual buffer; small-dilation layers go through a
DMA-built shifted tap-pair buffer (stride 10240) because tiny/unaligned
k-tile strides lock up the PE. The skip conv also runs DoubleRow (two
layers per pass) over a combined lo buffer, accumulating 4-layer groups in
PSUM with one drain per 1024 columns. The end convs (o1/o2) stay bf16:
that path has heavy cancellation and fp8 there alone costs 2.8e-2 error
(vs 6.4e-3 total for everything else fp8). PSUM is one unified
[128,1024]x4 ring; f-drains on Scalar, g-drain+multiply fused into one DVE
scalar_tensor_tensor, residual updates alternate DVE stt / Scalar+GpSimd,
and the residual-conv flush runs a 3-group software pipeline so DVE drain
latency never stalls the PE ring. The two global sequence edges
(first/last 1024 cols) are recomputed exactly on the host in numpy.
"""
import sys
if '/opt/trn_rl_repo' not in sys.path:
    sys.path.insert(0, '/opt/trn_rl_repo')
import numpy as np

B, L = 4, 65536
N_CORES = 8
L_CORE = L // N_CORES            # 8192
HALO_L, HALO_R = 1024, 1022
W = HALO_L + L_CORE + HALO_R     # 10238
CH = 512
NFIN = L_CORE // CH              # 16
GRP = 4                          # layers per deferred-skip group
NGRP = 20 // GRP
LOW = 10240                      # per-layer stride in the combined lo buffer
WP = 10240                       # half-stride of the shifted tap-pair buffer

# per-layer tap offsets (global layer l = blk*10 + i)
_OFFS = []
for _l in range(20):
    _i = _l % 10
    _d = 2 ** _i
    _OFFS.append((1, 0) if _i == 0 else (_d // 2, _d // 2))
# columns layer l must produce: [A[l], Bnd[l]) in window coords
_NLAFT = [sum(o[0] for o in _OFFS[_l + 1:]) for _l in range(20)]
_NRAFT = [sum(o[1] for o in _OFFS[_l + 1:]) for _l in range(20)]
A_COL = [HALO_L - _NLAFT[_l] for _l in range(20)]
B_COL = [HALO_L + L_CORE + _NRAFT[_l] for _l in range(20)]


def _chunk_groups(l):
    """Groups [(g0, [(c0, n), ...])] for layer l; each group shares one
    PSUM tile (<=1024 cols). Middle groups pair two aligned 512-col output
    chunks; left / right extensions are single chunks covering the
    receptive-field halo. Odd-width edge chunks are widened by one column
    into the adjacent covered region (double-computing identical values)."""
    a, b = A_COL[l], B_COL[l]
    # DR layers (dl >= 8) need 4-aligned chunk starts for the fp8 ifmap;
    # aligning down up to 3 cols below `a` stays within the producing
    # layer's valid range there. Small-dl layers keep 2-col widening.
    al = 4
    lefts = []
    x = HALO_L
    while x > a:
        x0 = max(a, x - CH)
        x0al = (x0 // 4) * 4  # may dip <=3 cols below `a`: still valid
        if x0al >= _OFFS[l][0]:
            x0 = x0al
        n = x - x0
        if n % al:
            n += al - n % al  # overlap into the chunk to the right
        lefts.append((x0, [(x0, n)]))
        x = x0
    lefts.reverse()
    rights = []
    x = HALO_L + L_CORE
    while x < b:
        x1 = min(b, x + CH)
        n = x1 - x
        c0 = x
        if n % al:
            c0 -= al - n % al  # overlap into the chunk to the left
            n += al - n % al
        rights.append((c0, [(c0, n)]))
        x = x1
    mids = [(HALO_L + k * 2 * CH,
             [(HALO_L + k * 2 * CH, CH), (HALO_L + k * 2 * CH + CH, CH)])
            for k in range(NFIN // 2)]
    return lefts + mids + rights

_NC_CACHE = {}


def _build_nc():
    key = ("v6",)
    if key in _NC_CACHE:
        return _NC_CACHE[key]
    import concourse.bacc as bacc
    import concourse.mybir as mybir
    import concourse.tile as tile
    from concourse.ap import AP as APc
    F32 = mybir.dt.float32
    BF16 = mybir.dt.bfloat16
    FP8 = mybir.dt.float8e4
    A = mybir.AluOpType
    AF = mybir.ActivationFunctionType
    DR = mybir.MatmulPerfMode.DoubleRow

    def ktile2(ap2, stride, n):
        """[p, 2, n] view of a 2D slice: k-tile dim with given col stride."""
        dims = list(ap2.ap)
        return APc(ap2.tensor, ap2.offset,
                   [list(dims[0]), [stride, 2], [1, n]])

    nc = bacc.Bacc("TRN2", target_bir_lowering=False, debug=False,
                   num_devices=N_CORES)
    x_d = nc.dram_tensor("xw", [4, W], BF16, kind="ExternalInput").ap()
    fgw_d = nc.dram_tensor("fgw", [20, 128, 512], FP8, kind="ExternalInput").ap()
    rw_d = nc.dram_tensor("rw", [20, 128, 128], FP8, kind="ExternalInput").ap()
    sw_d = nc.dram_tensor("sw", [NGRP, 128, GRP * 128], FP8,
                          kind="ExternalInput").ap()
    bias_d = nc.dram_tensor("biasw", [20, 128, 4], F32, kind="ExternalInput").ap()
    grpb_d = nc.dram_tensor("grpb", [128, NGRP], F32, kind="ExternalInput").ap()
    startw_d = nc.dram_tensor("startw", [4, 128], BF16, kind="ExternalInput").ap()
    startb_d = nc.dram_tensor("startb", [128, 1], F32, kind="ExternalInput").ap()
    c1w_d = nc.dram_tensor("c1w", [128, 1024], BF16, kind="ExternalInput").ap()
    b1w_d = nc.dram_tensor("b1w", [128, 2], F32, kind="ExternalInput").ap()
    c2w_d = nc.dram_tensor("c2w", [128, 512], BF16, kind="ExternalInput").ap()
    b2w_d = nc.dram_tensor("b2w", [128, 2], F32, kind="ExternalInput").ap()
    out_d = nc.dram_tensor("out", [4, 256, L_CORE], BF16,
                           kind="ExternalOutput").ap()

    with tile.TileContext(nc) as tc:
        with tc.tile_pool(name="big", bufs=1) as big, \
             tc.tile_pool(name="wts", bufs=2) as wts, \
             tc.tile_pool(name="cnk", bufs=3) as cnk:
            rA = big.tile([128, W], FP8, tag="rA")
            rB = big.tile([128, W], FP8, tag="rB")
            lo_all = big.tile([128, GRP * LOW], FP8, tag="lo_all")
            seg = big.tile([128, L_CORE], BF16, tag="seg")
            pairT = big.tile([128, 2 * WP], FP8, tag="pairT")
            xw_sb = big.tile([4, W], BF16, tag="xw_sb")
            startw = big.tile([4, 128], BF16, tag="startw")
            startb = big.tile([128, 1], F32, tag="startb")
            c1w = big.tile([128, 4 * 256], BF16, tag="c1w")
            b1w = big.tile([128, 2], F32, tag="b1w")
            c2w = big.tile([128, 512], BF16, tag="c2w")
            b2w = big.tile([128, 2], F32, tag="b2w")
            grpb = big.tile([128, NGRP], F32, tag="grpb")
            q0 = 0
            for qn in (512, 1024, 2048, 2048, 2048, 2558):
                nc.sync.dma_start(xw_sb[:, q0:q0 + qn], x_d[:, q0:q0 + qn])
                q0 += qn
            nc.sync.dma_start(startw[:, :], startw_d[:, :])
            nc.sync.dma_start(startb[:, :], startb_d[:, :])
            nc.sync.dma_start(c1w[:, :], c1w_d[:, :])
            nc.sync.dma_start(b1w[:, :], b1w_d[:, :])
            nc.sync.dma_start(c2w[:, :], c2w_d[:, :])
            nc.sync.dma_start(b2w[:, :], b2w_d[:, :])
            nc.sync.dma_start(grpb[:, :], grpb_d[:, :])

            with tc.tile_pool(name="psw", bufs=4, space="PSUM") as ps:
                # start conv: r0 over the full window [0, W), paired drains
                x = 0
                di = 0
                while x < W:
                    gn = min(2 * CH, W - x)
                    pt = ps.tile([128, 2 * CH], F32, tag="u")
                    s0 = x
                    while s0 < x + gn:
                        sn = min(CH, x + gn - s0)
                        nc.tensor.matmul(pt[:, s0 - x:s0 - x + sn],
                                         startw[:, :], xw_sb[:, s0:s0 + sn],
                                         start=True, stop=True)
                        s0 += sn
                    if di % 2 == 0:
                        nc.scalar.activation(rA[:, x:x + gn], pt[:, :gn],
                                             AF.Identity, bias=startb[:, 0:1])
                    else:
                        nc.vector.tensor_scalar(rA[:, x:x + gn], pt[:, :gn],
                                                startb[:, 0:1], 0.0,
                                                op0=A.add, op1=A.add)
                    di += 1
                    x += gn

                cur, nxt = rA, rB
                for grp in range(NGRP):
                    swt = wts.tile([128, GRP * 128], FP8, tag="sw")
                    nc.sync.dma_start(swt[:, :], sw_d[grp, :, :])
                    for li in range(GRP):
                        l = grp * GRP + li
                        offL, offR = _OFFS[l]
                        dl = offL + offR
                        lbase = li * LOW
                        fg = wts.tile([128, 512], FP8, tag="fg")
                        rw = wts.tile([128, 128], FP8, tag="rw")
                        bi = wts.tile([128, 4], F32, tag="bi")
                        nc.sync.dma_start(fg[:, :], fgw_d[l, :, :])
                        nc.sync.dma_start(rw[:, :], rw_d[l, :, :])
                        nc.sync.dma_start(bi[:, :], bias_d[l, :, :])
                        fw3 = ktile2(fg[:, 0:128], 128, 128)
                        gw3 = ktile2(fg[:, 256:384], 128, 128)

                        def flush_r(pend):
                            g0, gn, subs, parity = pend
                            if l >= 19:
                                return
                            rps = ps.tile([128, 2 * CH], F32, tag="u")
                            for (c0, n) in subs:
                                nc.tensor.matmul(
                                    rps[:, c0 - g0:c0 - g0 + n], rw[:, :],
                                    lo_all[:, lbase + c0:lbase + c0 + n],
                                    start=True, stop=True)
                            if parity == 0:
                                nc.vector.scalar_tensor_tensor(
                                    nxt[:, g0:g0 + gn], rps[:, :gn],
                                    bi[:, 2:3], cur[:, g0:g0 + gn],
                                    op0=A.add, op1=A.add)
                            else:
                                rtmp = cnk.tile([128, 2 * CH], BF16,
                                                tag="rtmp", bufs=2)
                                nc.scalar.activation(rtmp[:, :gn],
                                                     rps[:, :gn],
                                                     AF.Identity,
                                                     bias=bi[:, 2:3])
                                nc.gpsimd.tensor_tensor(
                                    nxt[:, g0:g0 + gn], rtmp[:, :gn],
                                    cur[:, g0:g0 + gn], op=A.add)

                        pends = []
                        for gi, (g0, subs) in enumerate(_chunk_groups(l)):
                            gn = sum(n for _, n in subs)
                            use_pair = dl < 8 and g0 >= offL and g0 % 4 == 0
                            if use_pair:
                                nc.sync.dma_start(
                                    pairT[:, g0:g0 + gn],
                                    cur[:, g0 - offL:g0 - offL + gn])
                                nc.sync.dma_start(
                                    pairT[:, WP + g0:WP + g0 + gn],
                                    cur[:, g0 + offR:g0 + offR + gn])
                            fps = ps.tile([128, 2 * CH], F32, tag="u")
                            gps = ps.tile([128, 2 * CH], F32, tag="u")
                            for (c0, n) in subs:
                                off = c0 - g0
                                if use_pair:
                                    foot = pairT[:, c0:c0 + n]
                                    rhs3 = APc(foot.tensor, foot.offset,
                                               [list(list(foot.ap)[0]),
                                                [WP, 2], [1, n]])
                                    nc.tensor.matmul(fps[:, off:off + n],
                                                     fw3, rhs3, start=True,
                                                     stop=True, perf_mode=DR)
                                    nc.tensor.matmul(gps[:, off:off + n],
                                                     gw3, rhs3, start=True,
                                                     stop=True, perf_mode=DR)
                                elif dl >= 8:
                                    # DoubleRow: both taps in one PE pass
                                    foot = cur[:, c0 - offL:c0 - offL + dl + n]
                                    rhs3 = APc(foot.tensor, foot.offset,
                                               [list(list(foot.ap)[0]),
                                                [dl, 2], [1, n]])
                                    nc.tensor.matmul(fps[:, off:off + n],
                                                     fw3, rhs3, start=True,
                                                     stop=True, perf_mode=DR)
                                    nc.tensor.matmul(gps[:, off:off + n],
                                                     gw3, rhs3, start=True,
                                                     stop=True, perf_mode=DR)
                                else:
                                    rhsL = cur[:, c0 - offL:c0 - offL + n]
                                    rhsR = cur[:, c0 + offR:c0 + offR + n]
                                    nc.tensor.matmul(fps[:, off:off + n],
                                                     fg[:, 0:128], rhsL,
                                                     start=True, stop=False)
                                    nc.tensor.matmul(fps[:, off:off + n],
                                                     fg[:, 128:256], rhsR,
                                                     start=False, stop=True)
                                    nc.tensor.matmul(gps[:, off:off + n],
                                                     fg[:, 256:384], rhsL,
                                                     start=True, stop=False)
                                    nc.tensor.matmul(gps[:, off:off + n],
                                                     fg[:, 384:512], rhsR,
                                                     start=False, stop=True)
                            fsb = cnk.tile([128, 2 * CH], BF16, tag="fsb")
                            nc.scalar.activation(fsb[:, :gn], fps[:, :gn],
                                                 AF.Identity, bias=bi[:, 0:1])
                            lo_ap = lo_all[:, lbase + g0:lbase + g0 + gn]
                            if gi % 8 == 2:
                                # split form: Act drains g, DVE multiplies at
                                # the 2x all-SBUF mode - relieves DVE
                                gsb = cnk.tile([128, 2 * CH], BF16,
                                               tag="gsb", bufs=2)
                                nc.scalar.activation(gsb[:, :gn], gps[:, :gn],
                                                     AF.Identity,
                                                     bias=bi[:, 1:2])
                                nc.vector.tensor_tensor(
                                    lo_ap, fsb[:, :gn], gsb[:, :gn],
                                    op=A.mult)
                            else:
                                nc.vector.scalar_tensor_tensor(
                                    lo_ap, gps[:, :gn], bi[:, 1:2],
                                    fsb[:, :gn], op0=A.add, op1=A.mult)
                            pends.append((g0, gn, subs, gi % 2))
                            if len(pends) > 3:
                                flush_r(pends.pop(0))
                        for pe_ in pends:
                            flush_r(pe_)
                        if l < 19:
                            cur, nxt = nxt, cur

                    # deferred skip: 4 layers accumulate in PSUM via fp8
                    # DoubleRow (two layers per pass), one drain per c-pair
                    for cp in range(NFIN // 2):
                        col0 = HALO_L + cp * 2 * CH
                        sp = ps.tile([128, 2 * CH], F32, tag="u")
                        for half in range(2):
                            c0 = col0 + half * CH
                            for li in (0, 2):
                                w3 = ktile2(swt[:, li * 128:(li + 1) * 128],
                                            128, 128)
                                ifm = ktile2(
                                    lo_all[:, li * LOW + c0:li * LOW + c0 + CH],
                                    LOW, CH)
                                nc.tensor.matmul(
                                    sp[:, half * CH:(half + 1) * CH],
                                    w3, ifm, start=(li == 0),
                                    stop=(li == 2), perf_mode=DR)
                        s_ap = seg[:, cp * 2 * CH:(cp + 1) * 2 * CH]
                        if grp == 0:
                            nc.scalar.activation(s_ap, sp[:, :], AF.Identity,
                                                 bias=grpb[:, 0:1])
                        else:
                            nc.vector.scalar_tensor_tensor(
                                s_ap, sp[:, :], grpb[:, grp:grp + 1], s_ap,
                                op0=A.add, op1=A.add)
                            if grp == NGRP - 1:
                                nc.scalar.activation(s_ap, s_ap, AF.Relu)

                # final stage: reuses the unified PSUM tag ring. Paired
                # across sequence chunks; all-bf16 for accuracy.
                # software-pipelined by one (cp, g) step.
                def emit_o1(cp, g):
                    rl0 = seg[:, cp * 2 * CH:cp * 2 * CH + CH]
                    rl1 = seg[:, cp * 2 * CH + CH:(cp + 1) * 2 * CH]
                    o1sb = []
                    for h in range(2):
                        o1ps = ps.tile([128, 2 * CH], F32, tag="u",
                                       name="o1ps")
                        wcol = (2 * g + h) * 128
                        nc.tensor.matmul(o1ps[:, 0:CH],
                                         c1w[:, wcol:wcol + 128], rl0,
                                         start=True, stop=True)
                        nc.tensor.matmul(o1ps[:, CH:2 * CH],
                                         c1w[:, wcol:wcol + 128], rl1,
                                         start=True, stop=True)
                        t = cnk.tile([128, 2 * CH], BF16, tag=f"o1sb_{h}",
                                     bufs=2)
                        if h == 0:
                            nc.scalar.activation(t[:, :], o1ps[:, :], AF.Relu,
                                                 bias=b1w[:, 0:1])
                        else:
                            nc.vector.tensor_scalar(t[:, :], o1ps[:, :],
                                                    b1w[:, 1:2], 0.0,
                                                    op0=A.add, op1=A.max)
                        o1sb.append(t)
                    return o1sb

                def emit_o2(cp, g, o1sb):
                    c0 = cp * 2 * CH
                    for h2 in range(2):
                        o2ps = ps.tile([128, 2 * CH], F32, tag="u",
                                       name="o2ps")
                        for half in range(2):
                            for h in range(2):
                                j = 2 * h + h2
                                nc.tensor.matmul(
                                    o2ps[:, half * CH:(half + 1) * CH],
                                    c2w[:, j * 128:(j + 1) * 128],
                                    o1sb[h][:, half * CH:(half + 1) * CH],
                                    start=(h == 0), stop=(h == 1))
                        o2sb = cnk.tile([128, 2 * CH], BF16, tag="o2sb")
                        if h2 == 0:
                            nc.scalar.activation(o2sb[:, :], o2ps[:, :],
                                                 AF.Identity,
                                                 bias=b2w[:, 0:1])
                        else:
                            nc.vector.tensor_scalar(
                                o2sb[:, :], o2ps[:, :], b2w[:, 1:2], 0.0,
                                op0=A.add, op1=A.add)
                        nc.sync.dma_start(
                            out_d[g, 128 * h2:128 * (h2 + 1), c0:c0 + 2 * CH],
                            o2sb[:, :])

                pend_f = None
                for cp in range(NFIN // 2):
                    for g in range(4):
                        o1sb = emit_o1(cp, g)
                        if pend_f is not None:
                            emit_o2(*pend_f)
                        pend_f = (cp, g, o1sb)
                emit_o2(*pend_f)
    nc.compile()
    _NC_CACHE[key] = nc
    return nc


def _host_arrays(inputs):
    """Builds the shared (core-independent) weight arrays."""
    import ml_dtypes
    BF = ml_dtypes.bfloat16
    F8 = ml_dtypes.float8_e4m3

    def make_bd(Wm):
        bd = np.zeros((128, 128), np.float32)
        for g in range(4):
            bd[32 * g:32 * g + 32, 32 * g:32 * g + 32] = Wm.T
        return bd

    fgw = np.zeros((20, 128, 512), np.float32)
    rw = np.zeros((20, 128, 128), np.float32)
    sw = np.zeros((NGRP, 128, GRP * 128), np.float32)
    biasw = np.zeros((20, 128, 4), np.float32)
    grpb = np.zeros((128, NGRP), np.float32)
    for l in range(20):
        blk, i = divmod(l, 10)
        fgw[l, :, 0:128] = make_bd(inputs['filt_w'][blk, i, :, :, 0])
        fgw[l, :, 128:256] = make_bd(inputs['filt_w'][blk, i, :, :, 1])
        fgw[l, :, 256:384] = make_bd(inputs['gate_w'][blk, i, :, :, 0])
        fgw[l, :, 384:512] = make_bd(inputs['gate_w'][blk, i, :, :, 1])
        rw[l] = make_bd(inputs['res_w'][blk, i, :, :, 0])
        g, li = divmod(l, GRP)
        sw[g, :, li * 128:(li + 1) * 128] = make_bd(inputs['skip_w'][blk, i, :, :, 0])
        biasw[l, :, 0] = np.tile(inputs['filt_b'][blk, i], 4)
        biasw[l, :, 1] = np.tile(inputs['gate_b'][blk, i], 4)
        biasw[l, :, 2] = np.tile(inputs['res_b'][blk, i], 4)
        grpb[:, g] += np.tile(inputs['skip_b'][blk, i], 4)
    startw = np.zeros((4, 128), np.float32)
    for g in range(4):
        startw[g, 32 * g:32 * g + 32] = inputs['w_start'][:, 0, 0]
    startb = np.tile(inputs['b_start'], 4).reshape(128, 1).astype(np.float32)
    c1w = np.zeros((4, 128, 256), np.float32)
    for g in range(4):
        for h in range(2):
            c1w[g, 32 * g:32 * g + 32, 128 * h:128 * h + 128] = \
                inputs['w_end1'][128 * h:128 * h + 128, :, 0].T
    b1w = np.stack([inputs['b_end1'][0:128], inputs['b_end1'][128:256]],
                   axis=1).astype(np.float32)
    c2w = np.zeros((128, 512), np.float32)
    for h in range(2):
        for h2 in range(2):
            c2w[:, (2 * h + h2) * 128:(2 * h + h2) * 128 + 128] = \
                inputs['w_end2'][128 * h2:128 * h2 + 128, 128 * h:128 * h + 128, 0].T
    b2w = np.stack([inputs['b_end2'][0:128], inputs['b_end2'][128:256]],
                   axis=1).astype(np.float32)
    # flatten to SBUF layout: col block (2g+h) holds group-g/out-half-h weights
    c1w_sb = np.ascontiguousarray(
        c1w.transpose(1, 0, 2).reshape(128, 1024))
    return dict(fgw=fgw.astype(F8), rw=rw.astype(F8), sw=sw.astype(F8),
                biasw=biasw, grpb=grpb, startw=startw.astype(BF),
                startb=startb, c1w=c1w_sb.astype(BF), b1w=b1w,
                c2w=c2w.astype(BF), b2w=b2w)


def _np_reference_strip(inputs, x_strip):
    """Exact fp32 reference on a short strip (true zero-padded edges)."""
    S = x_strip.shape[1]

    def layer_conv(r, Wm, b, offL, offR):
        rp = np.pad(r, ((0, 0), (0, 0), (offL, offR)))
        return (np.einsum('oc,bct->bot', Wm[:, :, 0], rp[:, :, 0:S]) +
                np.einsum('oc,bct->bot', Wm[:, :, 1],
                          rp[:, :, offL + offR:offL + offR + S]) +
                b[None, :, None])

    r = (inputs['w_start'][:, 0, 0][None, :, None] * x_strip[:, None, :] +
         inputs['b_start'][None, :, None])
    skip_total = np.zeros_like(r)
    for blk in range(2):
        skip = np.zeros_like(r)
        for i in range(10):
            d = 2 ** i
            offL, offR = (1, 0) if i == 0 else (d // 2, d // 2)
            f = layer_conv(r, inputs['filt_w'][blk, i], inputs['filt_b'][blk, i], offL, offR)
            g = layer_conv(r, inputs['gate_w'][blk, i], inputs['gate_b'][blk, i], offL, offR)
            lo = f * g
            skip = skip + np.einsum('oc,bct->bot', inputs['skip_w'][blk, i][:, :, 0], lo) \
                + inputs['skip_b'][blk, i][None, :, None]
            r = r + np.einsum('oc,bct->bot', inputs['res_w'][blk, i][:, :, 0], lo) \
                + inputs['res_b'][blk, i][None, :, None]
        skip_total = skip_total + skip
    out = np.maximum(skip_total, 0)
    out = np.maximum(np.einsum('oc,bct->bot', inputs['w_end1'][:, :, 0], out) +
                     inputs['b_end1'][None, :, None], 0)
    return (np.einsum('oc,bct->bot', inputs['w_end2'][:, :, 0], out) +
            inputs['b_end2'][None, :, None])


def run(trace=False, **inputs):
    from concourse.bass_utils import run_bass_kernel_spmd
    import ml_dtypes
    BF = ml_dtypes.bfloat16
    inputs = {k: np.ascontiguousarray(np.asarray(v, np.float32)) for k, v in inputs.items()}
    nc = _build_nc()
    shared = _host_arrays(inputs)
    x = inputs['x']  # [4, 1, L]
    in_maps = []
    for core in range(N_CORES):
        s = core * L_CORE
        xw = np.zeros((4, W), np.float32)
        lo_g, hi_g = s - HALO_L, s + L_CORE + HALO_R
        lo_c, hi_c = max(lo_g, 0), min(hi_g, L)
        xw[:, lo_c - lo_g: lo_c - lo_g + (hi_c - lo_c)] = x[:, 0, lo_c:hi_c]
        m = {"xw": xw.astype(BF)}
        m.update(shared)
        in_maps.append(m)
    res = run_bass_kernel_spmd(nc, in_maps, core_ids=list(range(N_CORES)),
                               trace=trace)
    out = np.zeros((B, 256, L), np.float32)
    for core in range(N_CORES):
        out[:, :, core * L_CORE:(core + 1) * L_CORE] = \
            np.asarray(res.results[core]["out"]).astype(np.float32)
    # host edge fix (device window edges differ from true sequence edges)
    STRIP = 2048
    left = _np_reference_strip(inputs, x[:, 0, :STRIP])
    out[:, :, :HALO_L] = left[:, :, :HALO_L]
    right = _np_reference_strip(inputs, x[:, 0, L - STRIP:])
    out[:, :, L - HALO_L:] = right[:, :, STRIP - HALO_L:]
    return out, res


def kernel(**inputs) -> np.ndarray:
    out, _ = run(trace=False, **inputs)
    return out


# revision 43
# speedup vs baseline: 1.1969x; 1.1969x over previous
"""BitwiseWavenet Trainium2 kernel: 8-core SPMD, sequence-parallel sharding.

Layout: 4 partition groups of 32 channels = the 4 batches; L split 8 ways
across cores, each core computing a halo-extended window of W=10238 samples.
All convs are PE matmuls with block-diagonal (per-group) weights.

v8: fp8e4m3 residual stream / filter/gate/res/skip weights / layer outputs
(lo). fp8 DoubleRow matmuls contract both dilation taps of f/g in one PE
pass: large-dilation layers (d>=8) read the two taps as a strided k-tile
pair directly from the residual buffer; small-dilation layers go through a
DMA-built shifted tap-pair buffer (stride 10240) because tiny/unaligned
k-tile strides lock up the PE. The skip conv also runs DoubleRow (two
layers per pass) over a combined lo buffer, accumulating 4-layer groups in
PSUM with one drain per 1024 columns. The end convs (o1/o2) stay bf16:
that path has heavy cancellation and fp8 there alone costs 2.8e-2 error
(vs 6.4e-3 total for everything else fp8). PSUM is one unified
[128,1024]x4 ring; f-drains on Scalar, g-drain+multiply fused into one DVE
scalar_tensor_tensor, residual updates alternate DVE stt / Scalar+GpSimd,
and the residual-conv flush runs a 3-group software pipeline so DVE drain
latency never stalls the PE ring. The two global sequence edges
(first/last 1024 cols) are recomputed exactly on the host in numpy.
"""
import sys
if '/opt/trn_rl_repo' not in sys.path:
    sys.path.insert(0, '/opt/trn_rl_repo')
import numpy as np

B, L = 4, 65536
N_CORES = 8
L_CORE = L // N_CORES            # 8192
HALO_L, HALO_R = 1024, 1022
W = HALO_L + L_CORE + HALO_R     # 10238
CH = 512
NFIN = L_CORE // CH              # 16
GRP = 4                          # layers per deferred-skip group
NGRP = 20 // GRP
LOW = 10240                      # per-layer stride in the combined lo buffer
WP = 10240                       # half-stride of the shifted tap-pair buffer

# per-layer tap offsets (global layer l = blk*10 + i)
_OFFS = []
for _l in range(20):
    _i = _l % 10
    _d = 2 ** _i
    _OFFS.append((1, 0) if _i == 0 else (_d // 2, _d // 2))
# columns layer l must produce: [A[l], Bnd[l]) in window coords
_NLAFT = [sum(o[0] for o in _OFFS[_l + 1:]) for _l in range(20)]
_NRAFT = [sum(o[1] for o in _OFFS[_l + 1:]) for _l in range(20)]
A_COL = [HALO_L - _NLAFT[_l] for _l in range(20)]
B_COL = [HALO_L + L_CORE + _NRAFT[_l] for _l in range(20)]


def _chunk_groups(l):
    """Groups [(g0, [(c0, n), ...])] for layer l; each group shares one
    PSUM tile (<=1024 cols). Middle groups pair two aligned 512-col output
    chunks; left / right extensions are single chunks covering the
    receptive-field halo. Odd-width edge chunks are widened by one column
    into the adjacent covered region (double-computing identical values)."""
    a, b = A_COL[l], B_COL[l]
    # DR layers (dl >= 8) need 4-aligned chunk starts for the fp8 ifmap;
    # aligning down up to 3 cols below `a` stays within the producing
    # layer's valid range there. Small-dl layers keep 2-col widening.
    al = 4
    lefts = []
    x = HALO_L
    while x > a:
        x0 = max(a, x - CH)
        x0al = (x0 // 4) * 4  # may dip <=3 cols below `a`: still valid
        if x0al >= _OFFS[l][0]:
            x0 = x0al
        n = x - x0
        if n % al:
            n += al - n % al  # overlap into the chunk to the right
        lefts.append((x0, [(x0, n)]))
        x = x0
    lefts.reverse()
    rights = []
    x = HALO_L + L_CORE
    while x < b:
        x1 = min(b, x + CH)
        n = x1 - x
        c0 = x
        if n % al:
            c0 -= al - n % al  # overlap into the chunk to the left
            n += al - n % al
        rights.append((c0, [(c0, n)]))
        x = x1
    mids = [(HALO_L + k * 2 * CH,
             [(HALO_L + k * 2 * CH, CH), (HALO_L + k * 2 * CH + CH, CH)])
            for k in range(NFIN // 2)]
    return lefts + mids + rights

_NC_CACHE = {}


def _build_nc():
    key = ("v6",)
    if key in _NC_CACHE:
        return _NC_CACHE[key]
    import concourse.bacc as bacc
    import concourse.mybir as mybir
    import concourse.tile as tile
    from concourse.ap import AP as APc
    F32 = mybir.dt.float32
    BF16 = mybir.dt.bfloat16
    FP8 = mybir.dt.float8e4
    A = mybir.AluOpType
    AF = mybir.ActivationFunctionType
    DR = mybir.MatmulPerfMode.DoubleRow

    def ktile2(ap2, stride, n):
        """[p, 2, n] view of a 2D slice: k-tile dim with given col stride."""
        dims = list(ap2.ap)
        return APc(ap2.tensor, ap2.offset,
                   [list(dims[0]), [stride, 2], [1, n]])

    nc = bacc.Bacc("TRN2", target_bir_lowering=False, debug=False,
                   num_devices=N_CORES)
    x_d = nc.dram_tensor("xw", [4, W], BF16, kind="ExternalInput").ap()
    fgw_d = nc.dram_tensor("fgw", [20, 128, 512], FP8, kind="ExternalInput").ap()
    rw_d = nc.dram_tensor("rw", [20, 128, 128], FP8, kind="ExternalInput").ap()
    sw_d = nc.dram_tensor("sw", [NGRP, 128, GRP * 128], FP8,
                          kind="ExternalInput").ap()
    bias_d = nc.dram_tensor("biasw", [20, 128, 4], F32, kind="ExternalInput").ap()
    grpb_d = nc.dram_tensor("grpb", [128, NGRP], F32, kind="ExternalInput").ap()
    startw_d = nc.dram_tensor("startw", [4, 128], BF16, kind="ExternalInput").ap()
    startb_d = nc.dram_tensor("startb", [128, 1], F32, kind="ExternalInput").ap()
    c1w_d = nc.dram_tensor("c1w", [128, 1024], BF16, kind="ExternalInput").ap()
    b1w_d = nc.dram_tensor("b1w", [128, 2], F32, kind="ExternalInput").ap()
    c2w_d = nc.dram_tensor("c2w", [128, 512], BF16, kind="ExternalInput").ap()
    b2w_d = nc.dram_tensor("b2w", [128, 2], F32, kind="ExternalInput").ap()
    out_d = nc.dram_tensor("out", [4, 256, L_CORE], BF16,
                           kind="ExternalOutput").ap()

    with tile.TileContext(nc) as tc:
        with tc.tile_pool(name="big", bufs=1) as big, \
             tc.tile_pool(name="wts", bufs=2) as wts, \
             tc.tile_pool(name="cnk", bufs=3) as cnk:
            rA = big.tile([128, W], FP8, tag="rA")
            rB = big.tile([128, W], FP8, tag="rB")
            lo_all = big.tile([128, GRP * LOW], FP8, tag="lo_all")
            seg = big.tile([128, L_CORE], BF16, tag="seg")
            pairT = big.tile([128, 2 * WP], FP8, tag="pairT")
            xw_sb = big.tile([4, W], BF16, tag="xw_sb")
            startw = big.tile([4, 128], BF16, tag="startw")
            startb = big.tile([128, 1], F32, tag="startb")
            c1w = big.tile([128, 4 * 256], BF16, tag="c1w")
            b1w = big.tile([128, 2], F32, tag="b1w")
            c2w = big.tile([128, 512], BF16, tag="c2w")
            b2w = big.tile([128, 2], F32, tag="b2w")
            grpb = big.tile([128, NGRP], F32, tag="grpb")
            q0 = 0
            for qn in (512, 1024, 2048, 2048, 2048, 2558):
                nc.sync.dma_start(xw_sb[:, q0:q0 + qn], x_d[:, q0:q0 + qn])
                q0 += qn
            nc.sync.dma_start(startw[:, :], startw_d[:, :])
            nc.sync.dma_start(startb[:, :], startb_d[:, :])
            nc.sync.dma_start(c1w[:, :], c1w_d[:, :])
            nc.sync.dma_start(b1w[:, :], b1w_d[:, :])
            nc.sync.dma_start(c2w[:, :], c2w_d[:, :])
            nc.sync.dma_start(b2w[:, :], b2w_d[:, :])
            nc.sync.dma_start(grpb[:, :], grpb_d[:, :])

            with tc.tile_pool(name="psw", bufs=4, space="PSUM") as ps:
                # start conv: r0 over the full window [0, W), paired drains
                x = 0
                di = 0
                while x < W:
                    gn = min(2 * CH, W - x)
                    pt = ps.tile([128, 2 * CH], F32, tag="u")
                    s0 = x
                    while s0 < x + gn:
                        sn = min(CH, x + gn - s0)
                        nc.tensor.matmul(pt[:, s0 - x:s0 - x + sn],
                                         startw[:, :], xw_sb[:, s0:s0 + sn],
                                         start=True, stop=True)
                        s0 += sn
                    if di % 2 == 0:
                        nc.scalar.activation(rA[:, x:x + gn], pt[:, :gn],
                                             AF.Identity, bias=startb[:, 0:1])
                    else:
                        nc.vector.tensor_scalar(rA[:, x:x + gn], pt[:, :gn],
                                                startb[:, 0:1], 0.0,
                                                op0=A.add, op1=A.add)
                    di += 1
                    x += gn

                cur, nxt = rA, rB
                for grp in range(NGRP):
                    swt = wts.tile([128, GRP * 128], FP8, tag="sw")
                    nc.sync.dma_start(swt[:, :], sw_d[grp, :, :])
                    for li in range(GRP):
                        l = grp * GRP + li
                        offL, offR = _OFFS[l]
                        dl = offL + offR
                        lbase = li * LOW
                        fg = wts.tile([128, 512], FP8, tag="fg")
                        rw = wts.tile([128, 128], FP8, tag="rw")
                        bi = wts.tile([128, 4], F32, tag="bi")
                        nc.sync.dma_start(fg[:, :], fgw_d[l, :, :])
                        nc.sync.dma_start(rw[:, :], rw_d[l, :, :])
                        nc.sync.dma_start(bi[:, :], bias_d[l, :, :])
                        fw3 = ktile2(fg[:, 0:128], 128, 128)
                        gw3 = ktile2(fg[:, 256:384], 128, 128)

                        def flush_r(pend):
                            g0, gn, subs, parity = pend
                            if l >= 19:
                                return
                            rps = ps.tile([128, 2 * CH], F32, tag="u")
                            for (c0, n) in subs:
                                nc.tensor.matmul(
                                    rps[:, c0 - g0:c0 - g0 + n], rw[:, :],
                                    lo_all[:, lbase + c0:lbase + c0 + n],
                                    start=True, stop=True)
                            if parity == 0:
                                nc.vector.scalar_tensor_tensor(
                                    nxt[:, g0:g0 + gn], rps[:, :gn],
                                    bi[:, 2:3], cur[:, g0:g0 + gn],
                                    op0=A.add, op1=A.add)
                            else:
                                rtmp = cnk.tile([128, 2 * CH], BF16,
                                                tag="rtmp", bufs=2)
                                nc.scalar.activation(rtmp[:, :gn],
                                                     rps[:, :gn],
                                                     AF.Identity,
                                                     bias=bi[:, 2:3])
                                nc.gpsimd.tensor_tensor(
                                    nxt[:, g0:g0 + gn], rtmp[:, :gn],
                                    cur[:, g0:g0 + gn], op=A.add)

                        pends = []
                        for gi, (g0, subs) in enumerate(_chunk_groups(l)):
                            gn = sum(n for _, n in subs)
                            use_pair = dl < 8 and g0 >= offL and g0 % 4 == 0
                            if use_pair:
                                nc.sync.dma_start(
                                    pairT[:, g0:g0 + gn],
                                    cur[:, g0 - offL:g0 - offL + gn])
                                nc.sync.dma_start(
                                    pairT[:, WP + g0:WP + g0 + gn],
                                    cur[:, g0 + offR:g0 + offR + gn])
                            fps = ps.tile([128, 2 * CH], F32, tag="u")
                            gps = ps.tile([128, 2 * CH], F32, tag="u")
                            for (c0, n) in subs:
                                off = c0 - g0
                                if use_pair:
                                    foot = pairT[:, c0:c0 + n]
                                    rhs3 = APc(foot.tensor, foot.offset,
                                               [list(list(foot.ap)[0]),
                                                [WP, 2], [1, n]])
                                    nc.tensor.matmul(fps[:, off:off + n],
                                                     fw3, rhs3, start=True,
                                                     stop=True, perf_mode=DR)
                                    nc.tensor.matmul(gps[:, off:off + n],
                                                     gw3, rhs3, start=True,
                                                     stop=True, perf_mode=DR)
                                elif dl >= 8:
                                    # DoubleRow: both taps in one PE pass
                                    foot = cur[:, c0 - offL:c0 - offL + dl + n]
                                    rhs3 = APc(foot.tensor, foot.offset,
                                               [list(list(foot.ap)[0]),
                                                [dl, 2], [1, n]])
                                    nc.tensor.matmul(fps[:, off:off + n],
                                                     fw3, rhs3, start=True,
                                                     stop=True, perf_mode=DR)
                                    nc.tensor.matmul(gps[:, off:off + n],
                                                     gw3, rhs3, start=True,
                                                     stop=True, perf_mode=DR)
                                else:
                                    rhsL = cur[:, c0 - offL:c0 - offL + n]
                                    rhsR = cur[:, c0 + offR:c0 + offR + n]
                                    nc.tensor.matmul(fps[:, off:off + n],
                                                     fg[:, 0:128], rhsL,
                                                     start=True, stop=False)
                                    nc.tensor.matmul(fps[:, off:off + n],
                                                     fg[:, 128:256], rhsR,
                                                     start=False, stop=True)
                                    nc.tensor.matmul(gps[:, off:off + n],
                                                     fg[:, 256:384], rhsL,
                                                     start=True, stop=False)
                                    nc.tensor.matmul(gps[:, off:off + n],
                                                     fg[:, 384:512], rhsR,
                                                     start=False, stop=True)
                            fsb = cnk.tile([128, 2 * CH], BF16, tag="fsb")
                            nc.scalar.activation(fsb[:, :gn], fps[:, :gn],
                                                 AF.Identity, bias=bi[:, 0:1])
                            nc.vector.scalar_tensor_tensor(
                                lo_all[:, lbase + g0:lbase + g0 + gn],
                                gps[:, :gn], bi[:, 1:2], fsb[:, :gn],
                                op0=A.add, op1=A.mult)
                            pends.append((g0, gn, subs, gi % 2))
                            if len(pends) > 3:
                                flush_r(pends.pop(0))
                        for pe_ in pends:
                            flush_r(pe_)
                        if l < 19:
                            cur, nxt = nxt, cur

                    # deferred skip: 4 layers accumulate in PSUM via fp8
                    # DoubleRow (two layers per pass), one drain per c-pair
                    for cp in range(NFIN // 2):
                        col0 = HALO_L + cp * 2 * CH
                        sp = ps.tile([128, 2 * CH], F32, tag="u")
                        for half in range(2):
                            c0 = col0 + half * CH
                            for li in (0, 2):
                                w3 = ktile2(swt[:, li * 128:(li + 1) * 128],
                                            128, 128)
                                ifm = ktile2(
                                    lo_all[:, li * LOW + c0:li * LOW + c0 + CH],
                                    LOW, CH)
                                nc.tensor.matmul(
                                    sp[:, half * CH:(half + 1) * CH],
                                    w3, ifm, start=(li == 0),
                                    stop=(li == 2), perf_mode=DR)
                        s_ap = seg[:, cp * 2 * CH:(cp + 1) * 2 * CH]
                        if grp == 0:
                            nc.scalar.activation(s_ap, sp[:, :], AF.Identity,
                                                 bias=grpb[:, 0:1])
                        else:
                            nc.vector.scalar_tensor_tensor(
                                s_ap, sp[:, :], grpb[:, grp:grp + 1], s_ap,
                                op0=A.add, op1=A.add)
                            if grp == NGRP - 1:
                                nc.scalar.activation(s_ap, s_ap, AF.Relu)

                # final stage: reuses the unified PSUM tag ring. Paired
                # across sequence chunks; all-bf16 for accuracy.
                # software-pipelined by one (cp, g) step.
                def emit_o1(cp, g):
                    rl0 = seg[:, cp * 2 * CH:cp * 2 * CH + CH]
                    rl1 = seg[:, cp * 2 * CH + CH:(cp + 1) * 2 * CH]
                    o1sb = []
                    for h in range(2):
                        o1ps = ps.tile([128, 2 * CH], F32, tag="u",
                                       name="o1ps")
                        wcol = (2 * g + h) * 128
                        nc.tensor.matmul(o1ps[:, 0:CH],
                                         c1w[:, wcol:wcol + 128], rl0,
                                         start=True, stop=True)
                        nc.tensor.matmul(o1ps[:, CH:2 * CH],
                                         c1w[:, wcol:wcol + 128], rl1,
                                         start=True, stop=True)
                        t = cnk.tile([128, 2 * CH], BF16, tag=f"o1sb_{h}",
                                     bufs=2)
                        if h == 0:
                            nc.scalar.activation(t[:, :], o1ps[:, :], AF.Relu,
                                                 bias=b1w[:, 0:1])
                        else:
                            nc.vector.tensor_scalar(t[:, :], o1ps[:, :],
                                                    b1w[:, 1:2], 0.0,
                                                    op0=A.add, op1=A.max)
                        o1sb.append(t)
                    return o1sb

                def emit_o2(cp, g, o1sb):
                    c0 = cp * 2 * CH
                    for h2 in range(2):
                        o2ps = ps.tile([128, 2 * CH], F32, tag="u",
                                       name="o2ps")
                        for half in range(2):
                            for h in range(2):
                                j = 2 * h + h2
                                nc.tensor.matmul(
                                    o2ps[:, half * CH:(half + 1) * CH],
                                    c2w[:, j * 128:(j + 1) * 128],
                                    o1sb[h][:, half * CH:(half + 1) * CH],
                                    start=(h == 0), stop=(h == 1))
                        o2sb = cnk.tile([128, 2 * CH], BF16, tag="o2sb")
                        if h2 == 0:
                            nc.scalar.activation(o2sb[:, :], o2ps[:, :],
                                                 AF.Identity,
                                                 bias=b2w[:, 0:1])
                        else:
                            nc.vector.tensor_scalar(
                                o2sb[:, :], o2ps[:, :], b2w[:, 1:2], 0.0,
                                op0=A.add, op1=A.add)
                        nc.sync.dma_start(
                            out_d[g, 128 * h2:128 * (h2 + 1), c0:c0 + 2 * CH],
                            o2sb[:, :])

                pend_f = None
                for cp in range(NFIN // 2):
                    for g in range(4):
                        o1sb = emit_o1(cp, g)
                        if pend_f is not None:
                            emit_o2(*pend_f)
                        pend_f = (cp, g, o1sb)
                emit_o2(*pend_f)
    nc.compile()
    _NC_CACHE[key] = nc
    return nc


def _host_arrays(inputs):
    """Builds the shared (core-independent) weight arrays."""
    import ml_dtypes
    BF = ml_dtypes.bfloat16
    F8 = ml_dtypes.float8_e4m3

    def make_bd(Wm):
        bd = np.zeros((128, 128), np.float32)
        for g in range(4):
            bd[32 * g:32 * g + 32, 32 * g:32 * g + 32] = Wm.T
        return bd

    fgw = np.zeros((20, 128, 512), np.float32)
    rw = np.zeros((20, 128, 128), np.float32)
    sw = np.zeros((NGRP, 128, GRP * 128), np.float32)
    biasw = np.zeros((20, 128, 4), np.float32)
    grpb = np.zeros((128, NGRP), np.float32)
    for l in range(20):
        blk, i = divmod(l, 10)
        fgw[l, :, 0:128] = make_bd(inputs['filt_w'][blk, i, :, :, 0])
        fgw[l, :, 128:256] = make_bd(inputs['filt_w'][blk, i, :, :, 1])
        fgw[l, :, 256:384] = make_bd(inputs['gate_w'][blk, i, :, :, 0])
        fgw[l, :, 384:512] = make_bd(inputs['gate_w'][blk, i, :, :, 1])
        rw[l] = make_bd(inputs['res_w'][blk, i, :, :, 0])
        g, li = divmod(l, GRP)
        sw[g, :, li * 128:(li + 1) * 128] = make_bd(inputs['skip_w'][blk, i, :, :, 0])
        biasw[l, :, 0] = np.tile(inputs['filt_b'][blk, i], 4)
        biasw[l, :, 1] = np.tile(inputs['gate_b'][blk, i], 4)
        biasw[l, :, 2] = np.tile(inputs['res_b'][blk, i], 4)
        grpb[:, g] += np.tile(inputs['skip_b'][blk, i], 4)
    startw = np.zeros((4, 128), np.float32)
    for g in range(4):
        startw[g, 32 * g:32 * g + 32] = inputs['w_start'][:, 0, 0]
    startb = np.tile(inputs['b_start'], 4).reshape(128, 1).astype(np.float32)
    c1w = np.zeros((4, 128, 256), np.float32)
    for g in range(4):
        for h in range(2):
            c1w[g, 32 * g:32 * g + 32, 128 * h:128 * h + 128] = \
                inputs['w_end1'][128 * h:128 * h + 128, :, 0].T
    b1w = np.stack([inputs['b_end1'][0:128], inputs['b_end1'][128:256]],
                   axis=1).astype(np.float32)
    c2w = np.zeros((128, 512), np.float32)
    for h in range(2):
        for h2 in range(2):
            c2w[:, (2 * h + h2) * 128:(2 * h + h2) * 128 + 128] = \
                inputs['w_end2'][128 * h2:128 * h2 + 128, 128 * h:128 * h + 128, 0].T
    b2w = np.stack([inputs['b_end2'][0:128], inputs['b_end2'][128:256]],
                   axis=1).astype(np.float32)
    # flatten to SBUF layout: col block (2g+h) holds group-g/out-half-h weights
    c1w_sb = np.ascontiguousarray(
        c1w.transpose(1, 0, 2).reshape(128, 1024))
    return dict(fgw=fgw.astype(F8), rw=rw.astype(F8), sw=sw.astype(F8),
                biasw=biasw, grpb=grpb, startw=startw.astype(BF),
                startb=startb, c1w=c1w_sb.astype(BF), b1w=b1w,
                c2w=c2w.astype(BF), b2w=b2w)


def _np_reference_strip(inputs, x_strip):
    """Exact fp32 reference on a short strip (true zero-padded edges)."""
    S = x_strip.shape[1]

    def layer_conv(r, Wm, b, offL, offR):
        rp = np.pad(r, ((0, 0), (0, 0), (offL, offR)))
        return (np.einsum('oc,bct->bot', Wm[:, :, 0], rp[:, :, 0:S]) +
                np.einsum('oc,bct->bot', Wm[:, :, 1],
                          rp[:, :, offL + offR:offL + offR + S]) +
                b[None, :, None])

    r = (inputs['w_start'][:, 0, 0][None, :, None] * x_strip[:, None, :] +
         inputs['b_start'][None, :, None])
    skip_total = np.zeros_like(r)
    for blk in range(2):
        skip = np.zeros_like(r)
        for i in range(10):
            d = 2 ** i
            offL, offR = (1, 0) if i == 0 else (d // 2, d // 2)
            f = layer_conv(r, inputs['filt_w'][blk, i], inputs['filt_b'][blk, i], offL, offR)
            g = layer_conv(r, inputs['gate_w'][blk, i], inputs['gate_b'][blk, i], offL, offR)
            lo = f * g
            skip = skip + np.einsum('oc,bct->bot', inputs['skip_w'][blk, i][:, :, 0], lo) \
                + inputs['skip_b'][blk, i][None, :, None]
            r = r + np.einsum('oc,bct->bot', inputs['res_w'][blk, i][:, :, 0], lo) \
                + inputs['res_b'][blk, i][None, :, None]
        skip_total = skip_total + skip
    out = np.maximum(skip_total, 0)
    out = np.maximum(np.einsum('oc,bct->bot', inputs['w_end1'][:, :, 0], out) +
                     inputs['b_end1'][None, :, None], 0)
    return (np.einsum('oc,bct->bot', inputs['w_end2'][:, :, 0], out) +
            inputs['b_end2'][None, :, None])


def run(trace=False, **inputs):
    from concourse.bass_utils import run_bass_kernel_spmd
    import ml_dtypes
    BF = ml_dtypes.bfloat16
    inputs = {k: np.ascontiguousarray(np.asarray(v, np.float32)) for k, v in inputs.items()}
    nc = _build_nc()
    shared = _host_arrays(inputs)
    x = inputs['x']  # [4, 1, L]
    in_maps = []
    for core in range(N_CORES):
        s = core * L_CORE
        xw = np.zeros((4, W), np.float32)
        lo_g, hi_g = s - HALO_L, s + L_CORE + HALO_R
        lo_c, hi_c = max(lo_g, 0), min(hi_g, L)
        xw[:, lo_c - lo_g: lo_c - lo_g + (hi_c - lo_c)] = x[:, 0, lo_c:hi_c]
        m = {"xw": xw.astype(BF)}
        m.update(shared)
        in_maps.append(m)
    res = run_bass_kernel_spmd(nc, in_maps, core_ids=list(range(N_CORES)),
                               trace=trace)
    out = np.zeros((B, 256, L), np.float32)
    for core in range(N_CORES):
        out[:, :, core * L_CORE:(core + 1) * L_CORE] = \
            np.asarray(res.results[core]["out"]).astype(np.float32)
    # host edge fix (device window edges differ from true sequence edges)
    STRIP = 2048
    left = _np_reference_strip(inputs, x[:, 0, :STRIP])
    out[:, :, :HALO_L] = left[:, :, :HALO_L]
    right = _np_reference_strip(inputs, x[:, 0, L - STRIP:])
    out[:, :, L - HALO_L:] = right[:, :, STRIP - HALO_L:]
    return out, res


def kernel(**inputs) -> np.ndarray:
    out, _ = run(trace=False, **inputs)
    return out
